# revision 1
# baseline (speedup 1.0000x reference)
"""Trainium2 Bass kernel for DeepDeltaResidualExpanded.

out = x + k_rms[..., :, None] * delta[..., None, :]
  k_rms = rmsnorm(k_in);  beta = 2*sigmoid(ctx @ bw.T + bb)
  proj = einsum('btd,btdv->btv', k_rms, x) * k_scale
  v    = sigmoid(v_in @ vw.T + vb) * 4
  delta = beta * (v - proj) * k_scale

Pure data parallel over B*T rows across 8 NeuronCores; the tiny
beta/v weights are replicated.  All contractions over D live in the
SBUF free dim and run as fused DVE multiply+reduce ops; the final
update is a fused (k * gamma_v) + x_v per DV lane, written in place.
"""

import numpy as np

B, T, D, DV = 4, 4096, 1024, 4
N_CORES = 8
ROWS = B * T
ROWS_PER_CORE = ROWS // N_CORES  # 2048
P = 128

K_EPS = 1e-05
V_SIG_SCALE = 4.0
# C = k_scale / sqrt(mean(k^2) + eps_rms) == 1/sqrt(sum(k^2) + D^2*eps_rms/D)
#   = 1/sqrt(sum_d k^2 + 1e-10)   (since eps_rms = K_EPS^2/D and D = 1024)
SQRT_BIAS = K_EPS * K_EPS  # 1e-10


def _build_nc(rows, repeat=1):
    """Build + compile the single-core Bass program for `rows` rows.

    repeat > 1 wraps the whole body in a HW loop that redoes identical
    work — only used by the benchmark harness to lift device time above
    host dispatch noise; results are unchanged (idempotent body).
    """
    import contextlib

    import concourse.bacc as bacc
    import concourse.mybir as mybir
    import concourse.tile as tile
    from concourse.bass import AP

    f32 = mybir.dt.float32
    Alu = mybir.AluOpType
    Act = mybir.ActivationFunctionType
    ntiles = rows // P
    assert rows % P == 0

    nc = bacc.Bacc("TRN2", target_bir_lowering=False, debug=False)

    x_d = nc.dram_tensor("x", [rows, D * DV], f32, kind="ExternalInput")
    k_d = nc.dram_tensor("k", [rows, D], f32, kind="ExternalInput")
    v_d = nc.dram_tensor("v", [rows, D], f32, kind="ExternalInput")
    c_d = nc.dram_tensor("c", [rows, D], f32, kind="ExternalInput")
    bw_d = nc.dram_tensor("bw", [1, D], f32, kind="ExternalInput")
    bb_d = nc.dram_tensor("bb", [1, 1], f32, kind="ExternalInput")
    vw_d = nc.dram_tensor("vw", [DV, D], f32, kind="ExternalInput")
    vb_d = nc.dram_tensor("vb", [1, DV], f32, kind="ExternalInput")
    y_d = nc.dram_tensor("y", [rows, D * DV], f32, kind="ExternalOutput")

    def pbcast(handle, shape):
        # Read the same DRAM bytes into all 128 partitions (step-0 AP).
        ap = handle.ap()
        return AP(tensor=ap.tensor, offset=ap.offset, ap=[[0, P], *ap.ap])

    with tile.TileContext(nc) as tc:
        with (
            tc.tile_pool(name="consts", bufs=1) as consts,
            tc.tile_pool(name="xp", bufs=3) as xp,
            tc.tile_pool(name="inp", bufs=3) as inp,
            tc.tile_pool(name="scrp", bufs=2) as scrp,
            tc.tile_pool(name="smallp", bufs=4) as smallp,
        ):
            bw_b = consts.tile([P, D], f32)
            nc.gpsimd.dma_start(out=bw_b[:], in_=pbcast(bw_d, None))
            vw_b = consts.tile([P, DV, D], f32)
            nc.gpsimd.dma_start(out=vw_b[:], in_=pbcast(vw_d, None))
            bb_b = consts.tile([P, 1], f32)
            nc.gpsimd.dma_start(out=bb_b[:], in_=pbcast(bb_d, None))
            vb_b = consts.tile([P, DV], f32)
            nc.gpsimd.dma_start(out=vb_b[:], in_=pbcast(vb_d, None))
            eps_t = consts.tile([P, 1], f32)
            nc.vector.memset(eps_t[:], SQRT_BIAS)

            loop_cm = (
                tc.For_i(0, repeat, 1) if repeat > 1 else contextlib.nullcontext()
            )
            with loop_cm:
                for i in range(ntiles):
                    r0 = i * P
                    x_t = xp.tile([P, D * DV], f32)
                    nc.sync.dma_start(out=x_t[:], in_=x_d.ap()[r0 : r0 + P, :])
                    k_t = inp.tile([P, D], f32, tag="k")
                    nc.sync.dma_start(out=k_t[:], in_=k_d.ap()[r0 : r0 + P, :])
                    v_t = inp.tile([P, D], f32, tag="v")
                    nc.sync.dma_start(out=v_t[:], in_=v_d.ap()[r0 : r0 + P, :])
                    c_t = inp.tile([P, D], f32, tag="c")
                    nc.sync.dma_start(out=c_t[:], in_=c_d.ap()[r0 : r0 + P, :])

                    x3 = x_t.rearrange("p (d v) -> p d v", v=DV)

                    # --- row stats: C = 1/sqrt(sum k^2 + 1e-10) (includes k_scale)
                    scr_a = scrp.tile([P, D], f32, tag="scr_a")
                    ms = smallp.tile([P, 1], f32, tag="ms")
                    nc.scalar.activation(scr_a[:], k_t[:], Act.Square, accum_out=ms[:])
                    s2 = smallp.tile([P, 1], f32, tag="s2")
                    nc.scalar.activation(s2[:], ms[:], Act.Sqrt, bias=eps_t[:])
                    cc = smallp.tile([P, 1], f32, tag="cc")
                    nc.vector.reciprocal(cc[:], s2[:])

                    # --- beta gate logits: sum_d ctx*bw
                    scr = scrp.tile([P, D], f32, tag="scr")
                    blog = smallp.tile([P, 1], f32, tag="blog")
                    nc.vector.scalar_tensor_tensor(
                        out=scr[:], in0=c_t[:], scalar=1.0, in1=bw_b[:],
                        op0=Alu.mult, op1=Alu.mult, accum_out=blog[:],
                    )
                    bsig = smallp.tile([P, 1], f32, tag="bsig")
                    nc.scalar.activation(bsig[:], blog[:], Act.Sigmoid, bias=bb_b[:])

                    # --- v gate logits: sum_d v_in*vw[j]
                    vlog = smallp.tile([P, DV], f32, tag="vlog")
                    for j in range(DV):
                        scr = scrp.tile([P, D], f32, tag="scr")
                        nc.vector.scalar_tensor_tensor(
                            out=scr[:], in0=v_t[:], scalar=1.0, in1=vw_b[:, j, :],
                            op0=Alu.mult, op1=Alu.mult,
                            accum_out=vlog[:, j : j + 1],
                        )
                    vlog2 = smallp.tile([P, DV], f32, tag="vlog2")
                    nc.vector.tensor_add(vlog2[:], vlog[:], vb_b[:])
                    vsig = smallp.tile([P, DV], f32, tag="vsig")
                    nc.scalar.activation(vsig[:], vlog2[:], Act.Sigmoid)

                    # --- pv[j] = C * sum_d k*x_j  (C folded in as the stt scalar)
                    pv = smallp.tile([P, DV], f32, tag="pv")
                    for j in range(DV):
                        scr = scrp.tile([P, D], f32, tag="scr")
                        nc.vector.scalar_tensor_tensor(
                            out=scr[:], in0=k_t[:], scalar=cc[:], in1=x3[:, :, j],
                            op0=Alu.mult, op1=Alu.mult,
                            accum_out=pv[:, j : j + 1],
                        )

                    # --- gamma[v] = 2*sigm(beta)*C * (4*sigm(v) - pv)
                    w = smallp.tile([P, DV], f32, tag="w")
                    nc.vector.scalar_tensor_tensor(
                        out=w[:], in0=vsig[:], scalar=V_SIG_SCALE, in1=pv[:],
                        op0=Alu.mult, op1=Alu.subtract,
                    )
                    bc = smallp.tile([P, 1], f32, tag="bc")
                    nc.vector.tensor_scalar(
                        out=bc[:], in0=bsig[:], scalar1=2.0, scalar2=cc[:],
                        op0=Alu.mult, op1=Alu.mult,
                    )
                    gamma = smallp.tile([P, DV], f32, tag="gamma")
                    nc.vector.tensor_scalar_mul(gamma[:], w[:], bc[:])

                    # --- out_v = k*gamma_v + x_v (in place), then store
                    for j in range(DV):
                        nc.vector.scalar_tensor_tensor(
                            out=x3[:, :, j], in0=k_t[:], scalar=gamma[:, j : j + 1],
                            in1=x3[:, :, j], op0=Alu.mult, op1=Alu.add,
                        )
                    # store via the second HWDGE engine (Activation) so queued
                    # stores never head-of-line block the load stream on SP
                    nc.scalar.dma_start(out=y_d.ap()[r0 : r0 + P, :], in_=x_t[:])

    nc.compile()
    return nc


_NC_CACHE = {}


def _get_nc(rows):
    if rows not in _NC_CACHE:
        _NC_CACHE[rows] = _build_nc(rows)
    return _NC_CACHE[rows]


def _shard_inputs(inputs):
    x = np.ascontiguousarray(inputs["x"], dtype=np.float32).reshape(ROWS, D * DV)
    k = np.ascontiguousarray(inputs["k_in"], dtype=np.float32).reshape(ROWS, D)
    v = np.ascontiguousarray(inputs["v_in"], dtype=np.float32).reshape(ROWS, D)
    c = np.ascontiguousarray(inputs["context"], dtype=np.float32).reshape(ROWS, D)
    bw = np.ascontiguousarray(inputs["beta_w"], dtype=np.float32).reshape(1, D)
    bb = np.ascontiguousarray(inputs["beta_b"], dtype=np.float32).reshape(1, 1)
    vw = np.ascontiguousarray(inputs["v_w"], dtype=np.float32).reshape(DV, D)
    vb = np.ascontiguousarray(inputs["v_b"], dtype=np.float32).reshape(1, DV)
    in_maps = []
    for core in range(N_CORES):
        sl = slice(core * ROWS_PER_CORE, (core + 1) * ROWS_PER_CORE)
        in_maps.append(
            {"x": x[sl], "k": k[sl], "v": v[sl], "c": c[sl],
             "bw": bw, "bb": bb, "vw": vw, "vb": vb}
        )
    return in_maps


def kernel_run(inputs, trace=False):
    """Returns (full output array, BassKernelResults)."""
    from concourse.bass_utils import run_bass_kernel_spmd

    nc = _get_nc(ROWS_PER_CORE)
    in_maps = _shard_inputs(inputs)
    res = run_bass_kernel_spmd(
        nc, in_maps, core_ids=list(range(N_CORES)), trace=trace
    )
    y = np.concatenate([res.results[c]["y"] for c in range(N_CORES)], axis=0)
    return y.reshape(B, T, D, DV), res


def kernel(**inputs):
    out, _ = kernel_run(inputs)
    return out

